# revision 18
# baseline (speedup 1.0000x reference)
"""Causal GQA self-attention on 8 Trainium2 NeuronCores.

Problem: B=2, S=2048, HIDDEN=2048, 16 q-heads, 4 kv-heads, head_dim=128, fp32 in/out.

Sharding: core c = 4*b + g  (b = batch, g = head-group).
Each core owns batch b and q-heads [4g, 4g+4) plus their shared kv-head g.

All on-chip operands are fp16 (host-cast); PSUM accumulation fp32.
Pipeline per core, interleaved so the AllGathers overlap attention compute:

  for c in 0..3:   (512-wide query chunk)
    proj(c):  QT/KT/VT for seq chunk c = W.T @ X.T chunk (output-major PSUM
              accumulation over the 16 hid k-tiles; 2 PSUM banks double-buffered)
              V[s,d] tiles via PE transpose.
    attn(c):  per head h: for key tile j<=4c+3: ST = KT_j.T @ QT_chunk,
              P = exp(ST/sqrt(d)+keybias) (ACT, fp16), causal mask mul on the
              diagonal tile (DVE), attnT += V_j.T @ P, l += ones.T @ P (PE).
              normalize attnT by 1/l -> f16 -> DMA to ag_in.
    AllGather(ag_in) across the 4 cores of the batch (runs on TOPSP/SDMA,
              overlapped with later chunks' proj+attn on the compute engines).
  for c in 0..3:   (only now; each waits only on its own AllGather)
    oproj(c): out slice [512 hid cols, chunk] = Wo_cols.T @ attnT_full(c).

Host gathers: out[b][:, 512g:512(g+1)] = core(b,g) outT.T.
"""

import numpy as np

HID = 2048
S = 2048
B = 2
NH = 16          # q heads total
D = 128          # head dim
G = 4            # head groups == cores per batch
HPG = NH // G    # q heads per group (4)
CH = 512         # seq chunk (free dim of moving operands)
NCH = S // CH    # 4 chunks
NKT = S // 128   # 16 key tiles
SCALE = 1.0 / float(np.sqrt(D))

_CACHED_NC = None


def _build_nc(sim_mode=False):
    import concourse.mybir as mybir
    import concourse.tile as tile
    from concourse import bacc

    F32 = mybir.dt.float32
    F16 = mybir.dt.float16
    Copy = mybir.ActivationFunctionType.Copy
    Exp = mybir.ActivationFunctionType.Exp

    nc = bacc.Bacc("TRN2", target_bir_lowering=False, debug=False,
                   num_devices=1 if sim_mode else 8)

    # ---- per-core input shards (fp16 except biases/keybias/out) ----
    xt = nc.declare_dram_parameter("xt", [HID, S], F16, isOutput=False)
    wq = nc.declare_dram_parameter("wq", [HID, HPG * D], F16, isOutput=False)
    wk = nc.declare_dram_parameter("wk", [HID, D], F16, isOutput=False)
    wv = nc.declare_dram_parameter("wv", [HID, D], F16, isOutput=False)
    wo = nc.declare_dram_parameter("wo", [HID, CH], F16, isOutput=False)
    bq = nc.declare_dram_parameter("bq", [D, HPG], F32, isOutput=False)
    bk = nc.declare_dram_parameter("bk", [D, 1], F32, isOutput=False)
    bv = nc.declare_dram_parameter("bv", [D, 1], F32, isOutput=False)
    bo = nc.declare_dram_parameter("bo", [D, HPG], F32, isOutput=False)
    maskt = nc.declare_dram_parameter("maskt", [128, 128], F16, isOutput=False)
    keybias = nc.declare_dram_parameter("keybias", [128, NKT], F32, isOutput=False)
    ident = nc.declare_dram_parameter("ident", [128, 128], F16, isOutput=False)
    ones = nc.declare_dram_parameter("ones", [128, 1], F16, isOutput=False)
    out = nc.declare_dram_parameter("out", [CH, S], F16, isOutput=True)

    groups = [[0, 1, 2, 3], [4, 5, 6, 7]]

    with tile.TileContext(nc) as tc:
        with (
            tc.tile_pool(name="persist", bufs=1) as persist,
            tc.tile_pool(name="xs", bufs=2) as xs,
            tc.tile_pool(name="mo", bufs=4) as mo,
            tc.tile_pool(name="pp", bufs=6) as pp,
            tc.tile_pool(name="att", bufs=3) as att,
            tc.tile_pool(name="rbp", bufs=3) as rbp,
            tc.tile_pool(name="vts", bufs=2) as vts,
            tc.tile_pool(name="ost", bufs=4) as ost,
            tc.tile_pool(name="ps_proj", bufs=2, space="PSUM") as ps_proj_pool,
            tc.tile_pool(name="ps", bufs=3, space="PSUM") as ps_pool,
            tc.tile_pool(name="ps_pv", bufs=2, space="PSUM") as ps_pv_pool,
            tc.tile_pool(name="ps_l", bufs=1, space="PSUM") as ps_l_pool,
            tc.tile_pool(name="dram", bufs=4, space="DRAM") as dram,
        ):
            # ---- persistent SBUF state ----
            qt_sb = persist.tile([128, HPG, S], F16)       # QT per head [d, h, s]
            kt_sb = persist.tile([128, S], F16)            # KT          [d, s]
            v_sb = persist.tile([128, NKT, D], F16)        # V           [s, j, d]
            wq_sb = persist.tile([128, NKT, HPG * D], F16)
            wk_sb = persist.tile([128, NKT, D], F16)
            wv_sb = persist.tile([128, NKT, D], F16)
            wo_sb = persist.tile([128, NKT, CH], F16)
            mask_sb = persist.tile([128, 128], F16)
            kb_sb = persist.tile([128, NKT], F32)
            ones_sb = persist.tile([128, 1], F16)
            idr_sb = persist.tile([128, 128], F16)
            bq_sb = persist.tile([D, HPG], F32)
            bk_sb = persist.tile([D, 1], F32)
            bv_sb = persist.tile([D, 1], F32)
            bo_sb = persist.tile([D, HPG], F32)

            nc.sync.dma_start(out=mask_sb, in_=maskt.ap())
            nc.sync.dma_start(out=kb_sb, in_=keybias.ap())
            nc.sync.dma_start(out=ones_sb, in_=ones.ap())
            nc.sync.dma_start(out=idr_sb, in_=ident.ap())
            nc.sync.dma_start(out=bq_sb, in_=bq.ap())
            nc.sync.dma_start(out=bk_sb, in_=bk.ap())
            nc.sync.dma_start(out=bv_sb, in_=bv.ap())
            nc.sync.dma_start(out=bo_sb, in_=bo.ap())

            # tiny warmup collective: absorbs rank launch skew + first-call
            # ncfw setup while the projections run, so the first real
            # AllGather runs at steady-state latency
            warm_in = dram.tile([128, 8], F16, name="warm_in")
            warm_out = dram.tile([G, 128, 8], F16, name="warm_out")
            nc.sync.dma_start(out=warm_in[:, 0:1], in_=ones_sb)
            if not sim_mode:
                nc.gpsimd.collective_compute(
                    "AllGather", mybir.AluOpType.bypass,
                    replica_groups=groups,
                    ins=[warm_in.opt()], outs=[warm_out.opt()],
                )

            # weights: (t p) n -> p t n so partition dim is hid-within-tile.
            # xt chunk 0 + per-k-tile weight transfers issue first and spread
            # across DMA queues so the first matmuls start within a few us.
            wq_r = wq.ap().rearrange("(t p) n -> p t n", p=128)
            wk_r = wk.ap().rearrange("(t p) n -> p t n", p=128)
            wv_r = wv.ap().rearrange("(t p) n -> p t n", p=128)
            wo_r = wo.ap().rearrange("(t p) n -> p t n", p=128)

            xt_r = xt.ap().rearrange("(t p) n -> p t n", p=128)

            def load_xt(c):
                xt_c = xs.tile([128, NKT, CH], F16, name="xt_c")
                sq = slice(c * CH, (c + 1) * CH)
                engs = [nc.sync, nc.scalar, nc.gpsimd, nc.sync]
                for u in range(4):
                    engs[u].dma_start(out=xt_c[:, 4 * u:4 * (u + 1), :],
                                      in_=xt_r[:, 4 * u:4 * (u + 1), sq])
                return xt_c

            xt_tiles = {0: load_xt(0)}
            for u in range(4):
                nc.gpsimd.dma_start(out=wq_sb[:, 4 * u:4 * (u + 1), :],
                                    in_=wq_r[:, 4 * u:4 * (u + 1), :])
            nc.gpsimd.dma_start(out=wk_sb, in_=wk_r)
            nc.gpsimd.dma_start(out=wv_sb, in_=wv_r)

            m_alls = []
            out_r = out.ap().rearrange("(t p) n -> p t n", p=128)

            def oproj(c):
                sq = slice(c * CH, (c + 1) * CH)
                m_all = m_alls[c]
                for t in range(HPG):
                    ps_o = ps_pool.tile([128, CH], F32, name="ps_any")
                    for ct in range(NKT):
                        nc.tensor.matmul(
                            ps_o, lhsT=wo_sb[:, ct, t * 128:(t + 1) * 128],
                            rhs=m_all[:, ct, :],
                            start=(ct == 0), stop=(ct == NKT - 1))
                    o_sb = ost.tile([128, CH], F16, name="o_sb")
                    nc.vector.tensor_scalar_add(o_sb, ps_o, bo_sb[:, t:t + 1])
                    eng = nc.sync if t % 2 == 0 else nc.gpsimd
                    eng.dma_start(out=out_r[:, t, sq], in_=o_sb)

            for c in range(NCH):
                sq = slice(c * CH, (c + 1) * CH)

                # ================= projections for chunk c =================
                # prefetch next chunk's xt ahead of this chunk's m_all loads
                if c + 1 < NCH:
                    xt_tiles[c + 1] = load_xt(c + 1)
                if c == 1:
                    nc.scalar.dma_start(out=wo_sb[:, :8, :], in_=wo_r[:, :8, :])
                    nc.scalar.dma_start(out=wo_sb[:, 8:, :], in_=wo_r[:, 8:, :])
                xt_c = xt_tiles[c]
                ps_k = ps_proj_pool.tile([128, CH], F32, name="ps_proj")
                for t in range(NKT):
                    nc.tensor.matmul(ps_k, lhsT=wk_sb[:, t, :], rhs=xt_c[:, t, :],
                                     start=(t == 0), stop=(t == NKT - 1))
                nc.vector.tensor_scalar_add(kt_sb[:, sq], ps_k, bk_sb)
                ps_v = ps_proj_pool.tile([128, CH], F32, name="ps_proj")
                for t in range(NKT):
                    nc.tensor.matmul(ps_v, lhsT=wv_sb[:, t, :], rhs=xt_c[:, t, :],
                                     start=(t == 0), stop=(t == NKT - 1))
                vt_t = vts.tile([128, CH], F16, name="vt_t")
                nc.vector.tensor_scalar_add(vt_t, ps_v, bv_sb)
                for u in range(4):
                    j = 4 * c + u
                    ps_tp = ps_pool.tile([128, 128], F16, name="ps_any")
                    nc.tensor.transpose(
                        ps_tp, vt_t[:, u * 128:(u + 1) * 128], idr_sb)
                    nc.scalar.activation(v_sb[:, j, :], ps_tp, Copy)
                for h in range(HPG):
                    ps_q = ps_proj_pool.tile([128, CH], F32, name="ps_proj")
                    for t in range(NKT):
                        nc.tensor.matmul(ps_q,
                                         lhsT=wq_sb[:, t, h * D:(h + 1) * D],
                                         rhs=xt_c[:, t, :],
                                         start=(t == 0), stop=(t == NKT - 1))
                    nc.vector.tensor_scalar_add(qt_sb[:, h, sq], ps_q,
                                                bq_sb[:, h:h + 1])

                # ================= attention for chunk c =================
                # heads processed in pairs: shared KT_j/V_j stationary loads,
                # rowsums packed into one PE pass via col-tiling (M=1 each at
                # col-groups 0 and 32), one reciprocal per pair.
                njt = 4 * c + 4  # causal: key tiles 0..4c+3
                ag_in = dram.tile([HPG, 128, CH], F16, name="ag_in")
                for pr in range(HPG // 2):
                    ha, hb = 2 * pr, 2 * pr + 1
                    ps_pv_a = ps_pv_pool.tile([128, CH], F32, name="ps_pv")
                    ps_pv_b = ps_pv_pool.tile([128, CH], F32, name="ps_pv")
                    ps_l = ps_l_pool.tile([33, CH], F32, name="ps_l")
                    for j in range(njt):
                        # key tile j only reaches queries >= 128*r into the
                        # chunk (r = j - 4c >= 0 on the diagonal)
                        r = max(0, j - 4 * c)
                        cs = slice(128 * r, CH)
                        qs_ = slice(c * CH + 128 * r, (c + 1) * CH)
                        kjt = kt_sb[:, j * 128:(j + 1) * 128]
                        ps_s_a = ps_pool.tile([128, CH], F32, name="ps_any")
                        ps_s_b = ps_pool.tile([128, CH], F32, name="ps_any")
                        nc.tensor.matmul(ps_s_a[:, cs], lhsT=kjt,
                                         rhs=qt_sb[:, ha, qs_],
                                         start=True, stop=True)
                        nc.tensor.matmul(ps_s_b[:, cs], lhsT=kjt,
                                         rhs=qt_sb[:, hb, qs_],
                                         start=True, stop=True)
                        p_a = pp.tile([128, CH], F16, name="p_a")
                        p_b = pp.tile([128, CH], F16, name="p_b")
                        nc.scalar.activation(p_a[:, cs], ps_s_a[:, cs], Exp,
                                             scale=SCALE, bias=kb_sb[:, j:j + 1])
                        nc.scalar.activation(p_b[:, cs], ps_s_b[:, cs], Exp,
                                             scale=SCALE, bias=kb_sb[:, j:j + 1])
                        if j >= 4 * c:
                            # triangular mask on the 128-wide diagonal block
                            ds = slice(128 * r, 128 * (r + 1))
                            nc.vector.tensor_mul(p_a[:, ds], p_a[:, ds], mask_sb)
                            nc.vector.tensor_mul(p_b[:, ds], p_b[:, ds], mask_sb)
                        st, sp = (j == 0), (j == njt - 1)
                        nc.tensor.matmul(ps_pv_a[:, cs], lhsT=v_sb[:, j, :],
                                         rhs=p_a[:, cs], start=st, stop=sp)
                        nc.tensor.matmul(ps_pv_b[:, cs], lhsT=v_sb[:, j, :],
                                         rhs=p_b[:, cs], start=st, stop=sp)
                        # two interleaved rowsum accumulation groups share one
                        # PSUM bank (partitions 0 / 32); col-tiled so the two
                        # M=1 matmuls run concurrently in the PE array.
                        nc.tensor.matmul(ps_l[0:1, cs], lhsT=ones_sb,
                                         rhs=p_a[:, cs], start=st, stop=sp,
                                         tile_position=(0, 0),
                                         skip_group_check=True)
                        nc.tensor.matmul(ps_l[32:33, cs], lhsT=ones_sb,
                                         rhs=p_b[:, cs], start=st, stop=sp,
                                         tile_position=(0, 32),
                                         skip_group_check=True)
                    # partition_broadcast requires its input at partition 0,
                    # so each reciprocal lands in its own partition-0 tile
                    rl_a = rbp.tile([1, CH], F32, name="rl_a")
                    rl_b = rbp.tile([1, CH], F32, name="rl_b")
                    nc.vector.reciprocal(rl_a, ps_l[0:1, :])
                    nc.vector.reciprocal(rl_b, ps_l[32:33, :])
                    rb_a = rbp.tile([128, CH], F32, name="rb_a")
                    rb_b = rbp.tile([128, CH], F32, name="rb_b")
                    nc.gpsimd.partition_broadcast(rb_a, rl_a, channels=128)
                    nc.gpsimd.partition_broadcast(rb_b, rl_b, channels=128)
                    at_a = att.tile([128, CH], F16, name="at_sb")
                    at_b = att.tile([128, CH], F16, name="at_sb")
                    nc.vector.tensor_mul(at_a, ps_pv_a, rb_a)
                    nc.vector.tensor_mul(at_b, ps_pv_b, rb_b)
                    nc.sync.dma_start(out=ag_in[ha], in_=at_a)
                    nc.sync.dma_start(out=ag_in[hb], in_=at_b)

                ag_out = dram.tile([G, HPG, 128, CH], F16, name="ag_out")
                if sim_mode:
                    # stand-in for the AllGather with equivalent local IO
                    for g in range(G):
                        nc.sync.dma_start(out=ag_out[g], in_=ag_in[:])
                else:
                    nc.gpsimd.collective_compute(
                        "AllGather", mybir.AluOpType.bypass,
                        replica_groups=groups,
                        ins=[ag_in.opt()], outs=[ag_out.opt()],
                    )
                m_all = mo.tile([128, NKT, CH], F16, name="m_all")
                ag_r = ag_out.rearrange("g h p n -> p (g h) n")
                nc.scalar.dma_start(out=m_all[:, :8, :], in_=ag_r[:, :8, :])
                nc.gpsimd.dma_start(out=m_all[:, 8:, :], in_=ag_r[:, 8:, :])
                m_alls.append(m_all)

            # ================= O-projections =================
            for c in range(NCH):
                oproj(c)

    nc.compile()
    return nc


def _host_consts():
    # triangular mask for the diagonal key tile of each 128-block:
    # mask[p, col] = 1.0 iff col >= p  (p = key pos, col = query pos in block)
    col = np.arange(128)[None, :]
    p = np.arange(128)[:, None]
    maskt = (col >= p).astype(np.float16)
    ident = np.eye(128, dtype=np.float16)
    ones = np.ones((128, 1), dtype=np.float16)
    return maskt, ident, ones


def kernel(hidden_states, attention_mask, Wq, bq, Wk, bk, Wv, bv, Wo, bo):
    from concourse.bass_utils import run_bass_kernel_spmd

    global _CACHED_NC
    if _CACHED_NC is None:
        _CACHED_NC = _build_nc()
    nc = _CACHED_NC

    X = np.asarray(hidden_states, dtype=np.float32).astype(np.float16)
    am = np.asarray(attention_mask).astype(np.float32)  # [B, S] key mask
    Wq16 = np.asarray(Wq, np.float32).astype(np.float16)
    Wk16 = np.asarray(Wk, np.float32).astype(np.float16)
    Wv16 = np.asarray(Wv, np.float32).astype(np.float16)
    Wo16 = np.asarray(Wo, np.float32).astype(np.float16)
    maskt, ident, ones = _host_consts()

    in_maps = []
    for c in range(8):
        b, g = divmod(c, G)
        qs = slice(g * HPG * D, (g + 1) * HPG * D)   # q-head cols of group g
        ks = slice(g * D, (g + 1) * D)               # kv-head cols of group g
        in_maps.append({
            "xt": np.ascontiguousarray(X[b].T),
            "wq": np.ascontiguousarray(Wq16[:, qs]),
            "wk": np.ascontiguousarray(Wk16[:, ks]),
            "wv": np.ascontiguousarray(Wv16[:, ks]),
            "wo": np.ascontiguousarray(Wo16[:, qs]),  # hid cols [512g, 512g+512)
            "bq": np.ascontiguousarray(
                np.asarray(bq, np.float32)[qs].reshape(HPG, D).T),
            "bk": np.asarray(bk, np.float32)[ks].reshape(D, 1).copy(),
            "bv": np.asarray(bv, np.float32)[ks].reshape(D, 1).copy(),
            "bo": np.ascontiguousarray(
                np.asarray(bo, np.float32)[qs].reshape(HPG, D).T),
            "maskt": maskt.copy(),
            "keybias": np.ascontiguousarray(
                ((1.0 - am[b]) * -10000.0).astype(np.float32)
                .reshape(NKT, 128).T),
            "ident": ident.copy(),
            "ones": ones.copy(),
        })

    global _last_in_maps
    _last_in_maps = in_maps
    res = run_bass_kernel_spmd(nc, in_maps, core_ids=list(range(8)))
    out = np.empty((B, S, HID), dtype=np.float32)
    for c in range(8):
        b, g = divmod(c, G)
        out[b][:, g * CH:(g + 1) * CH] = res.results[c]["out"].T.astype(np.float32)
    return out


# revision 19
# speedup vs baseline: 1.0395x; 1.0395x over previous
"""Causal GQA self-attention on 8 Trainium2 NeuronCores.

Problem: B=2, S=2048, HIDDEN=2048, 16 q-heads, 4 kv-heads, head_dim=128, fp32 in/out.

Sharding: core c = 4*b + g  (b = batch, g = head-group).
Each core owns batch b and q-heads [4g, 4g+4) plus their shared kv-head g.

All on-chip operands are fp16 (host-cast); PSUM accumulation fp32.
Pipeline per core, interleaved so the AllGathers overlap attention compute:

  for c in 0..3:   (512-wide query chunk)
    proj(c):  QT/KT/VT for seq chunk c = W.T @ X.T chunk (output-major PSUM
              accumulation over the 16 hid k-tiles; 2 PSUM banks double-buffered)
              V[s,d] tiles via PE transpose.
    attn(c):  per head h: for key tile j<=4c+3: ST = KT_j.T @ QT_chunk,
              P = exp(ST/sqrt(d)+keybias) (ACT, fp16), causal mask mul on the
              diagonal tile (DVE), attnT += V_j.T @ P, l += ones.T @ P (PE).
              normalize attnT by 1/l -> f16 -> DMA to ag_in.
    AllGather(ag_in) across the 4 cores of the batch (runs on TOPSP/SDMA,
              overlapped with later chunks' proj+attn on the compute engines).
  for c in 0..3:   (only now; each waits only on its own AllGather)
    oproj(c): out slice [512 hid cols, chunk] = Wo_cols.T @ attnT_full(c).

Host gathers: out[b][:, 512g:512(g+1)] = core(b,g) outT.T.
"""

import numpy as np

HID = 2048
S = 2048
B = 2
NH = 16          # q heads total
D = 128          # head dim
G = 4            # head groups == cores per batch
HPG = NH // G    # q heads per group (4)
CH = 512         # seq chunk (free dim of moving operands)
NCH = S // CH    # 4 chunks
NKT = S // 128   # 16 key tiles
SCALE = 1.0 / float(np.sqrt(D))

_CACHED_NC = None


def _build_nc(sim_mode=False):
    import concourse.mybir as mybir
    import concourse.tile as tile
    from concourse import bacc

    F32 = mybir.dt.float32
    F16 = mybir.dt.float16
    Copy = mybir.ActivationFunctionType.Copy
    Exp = mybir.ActivationFunctionType.Exp

    nc = bacc.Bacc("TRN2", target_bir_lowering=False, debug=False,
                   num_devices=1 if sim_mode else 8)

    # ---- per-core input shards (fp16 except biases/keybias/out) ----
    xt = nc.declare_dram_parameter("xt", [HID, S], F16, isOutput=False)
    wq = nc.declare_dram_parameter("wq", [HID, HPG * D], F16, isOutput=False)
    wk = nc.declare_dram_parameter("wk", [HID, D], F16, isOutput=False)
    wv = nc.declare_dram_parameter("wv", [HID, D], F16, isOutput=False)
    wo = nc.declare_dram_parameter("wo", [HID, CH], F16, isOutput=False)
    bq = nc.declare_dram_parameter("bq", [D, HPG], F32, isOutput=False)
    bk = nc.declare_dram_parameter("bk", [D, 1], F32, isOutput=False)
    bv = nc.declare_dram_parameter("bv", [D, 1], F32, isOutput=False)
    bo = nc.declare_dram_parameter("bo", [D, HPG], F32, isOutput=False)
    maskt = nc.declare_dram_parameter("maskt", [128, 128], F16, isOutput=False)
    keybias = nc.declare_dram_parameter("keybias", [128, NKT], F32, isOutput=False)
    ident = nc.declare_dram_parameter("ident", [128, 128], F16, isOutput=False)
    ones = nc.declare_dram_parameter("ones", [128, 1], F16, isOutput=False)
    out = nc.declare_dram_parameter("out", [CH, S], F16, isOutput=True)

    groups = [[0, 1, 2, 3], [4, 5, 6, 7]]

    with tile.TileContext(nc) as tc:
        with (
            tc.tile_pool(name="persist", bufs=1) as persist,
            tc.tile_pool(name="xs", bufs=2) as xs,
            tc.tile_pool(name="mo", bufs=4) as mo,
            tc.tile_pool(name="pp", bufs=6) as pp,
            tc.tile_pool(name="att", bufs=3) as att,
            tc.tile_pool(name="rbp", bufs=3) as rbp,
            tc.tile_pool(name="vts", bufs=2) as vts,
            tc.tile_pool(name="ost", bufs=4) as ost,
            tc.tile_pool(name="ps_proj", bufs=2, space="PSUM") as ps_proj_pool,
            tc.tile_pool(name="ps", bufs=3, space="PSUM") as ps_pool,
            tc.tile_pool(name="ps_pv", bufs=2, space="PSUM") as ps_pv_pool,
            tc.tile_pool(name="ps_l", bufs=1, space="PSUM") as ps_l_pool,
            tc.tile_pool(name="dram", bufs=4, space="DRAM") as dram,
        ):
            # ---- persistent SBUF state ----
            qt_sb = persist.tile([128, HPG, S], F16)       # QT per head [d, h, s]
            kt_sb = persist.tile([128, S], F16)            # KT          [d, s]
            v_sb = persist.tile([128, NKT, D], F16)        # V           [s, j, d]
            wq_sb = persist.tile([128, NKT, HPG * D], F16)
            wk_sb = persist.tile([128, NKT, D], F16)
            wv_sb = persist.tile([128, NKT, D], F16)
            wo_sb = persist.tile([128, NKT, CH], F16)
            mask_sb = persist.tile([128, 128], F16)
            kb_sb = persist.tile([128, NKT], F32)
            ones_sb = persist.tile([128, 1], F16)
            idr_sb = persist.tile([128, 128], F16)
            bq_sb = persist.tile([D, HPG], F32)
            bk_sb = persist.tile([D, 1], F32)
            bv_sb = persist.tile([D, 1], F32)
            bo_sb = persist.tile([D, HPG], F32)

            nc.sync.dma_start(out=mask_sb, in_=maskt.ap())
            nc.sync.dma_start(out=kb_sb, in_=keybias.ap())
            nc.sync.dma_start(out=ones_sb, in_=ones.ap())
            nc.sync.dma_start(out=idr_sb, in_=ident.ap())
            nc.sync.dma_start(out=bq_sb, in_=bq.ap())
            nc.sync.dma_start(out=bk_sb, in_=bk.ap())
            nc.sync.dma_start(out=bv_sb, in_=bv.ap())
            nc.sync.dma_start(out=bo_sb, in_=bo.ap())

            # tiny warmup collective: absorbs rank launch skew + first-call
            # ncfw setup while the projections run, so the first real
            # AllGather runs at steady-state latency
            warm_in = dram.tile([128, 8], F16, name="warm_in")
            warm_out = dram.tile([G, 128, 8], F16, name="warm_out")
            nc.sync.dma_start(out=warm_in[:, 0:1], in_=ones_sb)
            if not sim_mode:
                nc.gpsimd.collective_compute(
                    "AllGather", mybir.AluOpType.bypass,
                    replica_groups=groups,
                    ins=[warm_in.opt()], outs=[warm_out.opt()],
                )

            # weights: (t p) n -> p t n so partition dim is hid-within-tile.
            # xt chunk 0 + per-k-tile weight transfers issue first and spread
            # across DMA queues so the first matmuls start within a few us.
            wq_r = wq.ap().rearrange("(t p) n -> p t n", p=128)
            wk_r = wk.ap().rearrange("(t p) n -> p t n", p=128)
            wv_r = wv.ap().rearrange("(t p) n -> p t n", p=128)
            wo_r = wo.ap().rearrange("(t p) n -> p t n", p=128)

            xt_r = xt.ap().rearrange("(t p) n -> p t n", p=128)

            def load_xt(c):
                xt_c = xs.tile([128, NKT, CH], F16, name="xt_c")
                sq = slice(c * CH, (c + 1) * CH)
                engs = [nc.sync, nc.scalar, nc.gpsimd, nc.sync]
                for u in range(4):
                    engs[u].dma_start(out=xt_c[:, 4 * u:4 * (u + 1), :],
                                      in_=xt_r[:, 4 * u:4 * (u + 1), sq])
                return xt_c

            xt_tiles = {0: load_xt(0)}
            for u in range(4):
                nc.gpsimd.dma_start(out=wq_sb[:, 4 * u:4 * (u + 1), :],
                                    in_=wq_r[:, 4 * u:4 * (u + 1), :])
            nc.gpsimd.dma_start(out=wk_sb, in_=wk_r)
            nc.gpsimd.dma_start(out=wv_sb, in_=wv_r)

            m_alls = []
            out_r = out.ap().rearrange("(t p) n -> p t n", p=128)

            def oproj(c):
                sq = slice(c * CH, (c + 1) * CH)
                m_all = m_alls[c]
                for t in range(HPG):
                    ps_o = ps_pool.tile([128, CH], F32, name="ps_any")
                    for ct in range(NKT):
                        nc.tensor.matmul(
                            ps_o, lhsT=wo_sb[:, ct, t * 128:(t + 1) * 128],
                            rhs=m_all[:, ct, :],
                            start=(ct == 0), stop=(ct == NKT - 1))
                    o_sb = ost.tile([128, CH], F16, name="o_sb")
                    nc.vector.tensor_scalar_add(o_sb, ps_o, bo_sb[:, t:t + 1])
                    eng = nc.sync if t % 2 == 0 else nc.gpsimd
                    eng.dma_start(out=out_r[:, t, sq], in_=o_sb)

            for c in range(NCH):
                sq = slice(c * CH, (c + 1) * CH)

                # ================= projections for chunk c =================
                # prefetch next chunk's xt ahead of this chunk's m_all loads
                if c + 1 < NCH:
                    xt_tiles[c + 1] = load_xt(c + 1)
                if c == 1:
                    nc.scalar.dma_start(out=wo_sb[:, :8, :], in_=wo_r[:, :8, :])
                    nc.scalar.dma_start(out=wo_sb[:, 8:, :], in_=wo_r[:, 8:, :])
                xt_c = xt_tiles[c]
                for h in range(HPG):
                    ps_q = ps_proj_pool.tile([128, CH], F32, name="ps_proj")
                    for t in range(NKT):
                        nc.tensor.matmul(ps_q,
                                         lhsT=wq_sb[:, t, h * D:(h + 1) * D],
                                         rhs=xt_c[:, t, :],
                                         start=(t == 0), stop=(t == NKT - 1))
                    nc.vector.tensor_scalar_add(qt_sb[:, h, sq], ps_q,
                                                bq_sb[:, h:h + 1])
                ps_k = ps_proj_pool.tile([128, CH], F32, name="ps_proj")
                for t in range(NKT):
                    nc.tensor.matmul(ps_k, lhsT=wk_sb[:, t, :], rhs=xt_c[:, t, :],
                                     start=(t == 0), stop=(t == NKT - 1))
                nc.vector.tensor_scalar_add(kt_sb[:, sq], ps_k, bk_sb)
                ps_v = ps_proj_pool.tile([128, CH], F32, name="ps_proj")
                for t in range(NKT):
                    nc.tensor.matmul(ps_v, lhsT=wv_sb[:, t, :], rhs=xt_c[:, t, :],
                                     start=(t == 0), stop=(t == NKT - 1))
                vt_t = vts.tile([128, CH], F16, name="vt_t")
                nc.vector.tensor_scalar_add(vt_t, ps_v, bv_sb)
                for u in range(4):
                    j = 4 * c + u
                    ps_tp = ps_pool.tile([128, 128], F16, name="ps_any")
                    nc.tensor.transpose(
                        ps_tp, vt_t[:, u * 128:(u + 1) * 128], idr_sb)
                    nc.scalar.activation(v_sb[:, j, :], ps_tp, Copy)

                # ================= attention for chunk c =================
                # heads processed in pairs: shared KT_j/V_j stationary loads,
                # rowsums packed into one PE pass via col-tiling (M=1 each at
                # col-groups 0 and 32), one reciprocal per pair.
                njt = 4 * c + 4  # causal: key tiles 0..4c+3
                ag_in = dram.tile([HPG, 128, CH], F16, name="ag_in")
                for pr in range(HPG // 2):
                    ha, hb = 2 * pr, 2 * pr + 1
                    ps_pv_a = ps_pv_pool.tile([128, CH], F32, name="ps_pv")
                    ps_pv_b = ps_pv_pool.tile([128, CH], F32, name="ps_pv")
                    ps_l = ps_l_pool.tile([33, CH], F32, name="ps_l")
                    for j in range(njt):
                        # key tile j only reaches queries >= 128*r into the
                        # chunk (r = j - 4c >= 0 on the diagonal)
                        r = max(0, j - 4 * c)
                        cs = slice(128 * r, CH)
                        qs_ = slice(c * CH + 128 * r, (c + 1) * CH)
                        kjt = kt_sb[:, j * 128:(j + 1) * 128]
                        ps_s_a = ps_pool.tile([128, CH], F32, name="ps_any")
                        ps_s_b = ps_pool.tile([128, CH], F32, name="ps_any")
                        nc.tensor.matmul(ps_s_a[:, cs], lhsT=kjt,
                                         rhs=qt_sb[:, ha, qs_],
                                         start=True, stop=True)
                        nc.tensor.matmul(ps_s_b[:, cs], lhsT=kjt,
                                         rhs=qt_sb[:, hb, qs_],
                                         start=True, stop=True)
                        p_a = pp.tile([128, CH], F16, name="p_a")
                        p_b = pp.tile([128, CH], F16, name="p_b")
                        nc.scalar.activation(p_a[:, cs], ps_s_a[:, cs], Exp,
                                             scale=SCALE, bias=kb_sb[:, j:j + 1])
                        nc.scalar.activation(p_b[:, cs], ps_s_b[:, cs], Exp,
                                             scale=SCALE, bias=kb_sb[:, j:j + 1])
                        if j >= 4 * c:
                            # triangular mask on the 128-wide diagonal block
                            ds = slice(128 * r, 128 * (r + 1))
                            nc.vector.tensor_mul(p_a[:, ds], p_a[:, ds], mask_sb)
                            nc.vector.tensor_mul(p_b[:, ds], p_b[:, ds], mask_sb)
                        st, sp = (j == 0), (j == njt - 1)
                        nc.tensor.matmul(ps_pv_a[:, cs], lhsT=v_sb[:, j, :],
                                         rhs=p_a[:, cs], start=st, stop=sp)
                        nc.tensor.matmul(ps_pv_b[:, cs], lhsT=v_sb[:, j, :],
                                         rhs=p_b[:, cs], start=st, stop=sp)
                        # two interleaved rowsum accumulation groups share one
                        # PSUM bank (partitions 0 / 32); col-tiled so the two
                        # M=1 matmuls run concurrently in the PE array.
                        nc.tensor.matmul(ps_l[0:1, cs], lhsT=ones_sb,
                                         rhs=p_a[:, cs], start=st, stop=sp,
                                         tile_position=(0, 0),
                                         skip_group_check=True)
                        nc.tensor.matmul(ps_l[32:33, cs], lhsT=ones_sb,
                                         rhs=p_b[:, cs], start=st, stop=sp,
                                         tile_position=(0, 32),
                                         skip_group_check=True)
                    # partition_broadcast requires its input at partition 0,
                    # so each reciprocal lands in its own partition-0 tile
                    rl_a = rbp.tile([1, CH], F32, name="rl_a")
                    rl_b = rbp.tile([1, CH], F32, name="rl_b")
                    nc.vector.reciprocal(rl_a, ps_l[0:1, :])
                    nc.vector.reciprocal(rl_b, ps_l[32:33, :])
                    rb_a = rbp.tile([128, CH], F32, name="rb_a")
                    rb_b = rbp.tile([128, CH], F32, name="rb_b")
                    nc.gpsimd.partition_broadcast(rb_a, rl_a, channels=128)
                    nc.gpsimd.partition_broadcast(rb_b, rl_b, channels=128)
                    at_a = att.tile([128, CH], F16, name="at_sb")
                    at_b = att.tile([128, CH], F16, name="at_sb")
                    nc.vector.tensor_mul(at_a, ps_pv_a, rb_a)
                    nc.vector.tensor_mul(at_b, ps_pv_b, rb_b)
                    nc.sync.dma_start(out=ag_in[ha], in_=at_a)
                    nc.sync.dma_start(out=ag_in[hb], in_=at_b)

                ag_out = dram.tile([G, HPG, 128, CH], F16, name="ag_out")
                if sim_mode:
                    # stand-in for the AllGather with equivalent local IO
                    for g in range(G):
                        nc.sync.dma_start(out=ag_out[g], in_=ag_in[:])
                else:
                    nc.gpsimd.collective_compute(
                        "AllGather", mybir.AluOpType.bypass,
                        replica_groups=groups,
                        ins=[ag_in.opt()], outs=[ag_out.opt()],
                    )
                m_all = mo.tile([128, NKT, CH], F16, name="m_all")
                ag_r = ag_out.rearrange("g h p n -> p (g h) n")
                nc.scalar.dma_start(out=m_all[:, :8, :], in_=ag_r[:, :8, :])
                nc.gpsimd.dma_start(out=m_all[:, 8:, :], in_=ag_r[:, 8:, :])
                m_alls.append(m_all)

            # ================= O-projections =================
            for c in range(NCH):
                oproj(c)

    nc.compile()
    return nc


def _host_consts():
    # triangular mask for the diagonal key tile of each 128-block:
    # mask[p, col] = 1.0 iff col >= p  (p = key pos, col = query pos in block)
    col = np.arange(128)[None, :]
    p = np.arange(128)[:, None]
    maskt = (col >= p).astype(np.float16)
    ident = np.eye(128, dtype=np.float16)
    ones = np.ones((128, 1), dtype=np.float16)
    return maskt, ident, ones


def kernel(hidden_states, attention_mask, Wq, bq, Wk, bk, Wv, bv, Wo, bo):
    from concourse.bass_utils import run_bass_kernel_spmd

    global _CACHED_NC
    if _CACHED_NC is None:
        _CACHED_NC = _build_nc()
    nc = _CACHED_NC

    X = np.asarray(hidden_states, dtype=np.float32).astype(np.float16)
    am = np.asarray(attention_mask).astype(np.float32)  # [B, S] key mask
    Wq16 = np.asarray(Wq, np.float32).astype(np.float16)
    Wk16 = np.asarray(Wk, np.float32).astype(np.float16)
    Wv16 = np.asarray(Wv, np.float32).astype(np.float16)
    Wo16 = np.asarray(Wo, np.float32).astype(np.float16)
    maskt, ident, ones = _host_consts()

    in_maps = []
    for c in range(8):
        b, g = divmod(c, G)
        qs = slice(g * HPG * D, (g + 1) * HPG * D)   # q-head cols of group g
        ks = slice(g * D, (g + 1) * D)               # kv-head cols of group g
        in_maps.append({
            "xt": np.ascontiguousarray(X[b].T),
            "wq": np.ascontiguousarray(Wq16[:, qs]),
            "wk": np.ascontiguousarray(Wk16[:, ks]),
            "wv": np.ascontiguousarray(Wv16[:, ks]),
            "wo": np.ascontiguousarray(Wo16[:, qs]),  # hid cols [512g, 512g+512)
            "bq": np.ascontiguousarray(
                np.asarray(bq, np.float32)[qs].reshape(HPG, D).T),
            "bk": np.asarray(bk, np.float32)[ks].reshape(D, 1).copy(),
            "bv": np.asarray(bv, np.float32)[ks].reshape(D, 1).copy(),
            "bo": np.ascontiguousarray(
                np.asarray(bo, np.float32)[qs].reshape(HPG, D).T),
            "maskt": maskt.copy(),
            "keybias": np.ascontiguousarray(
                ((1.0 - am[b]) * -10000.0).astype(np.float32)
                .reshape(NKT, 128).T),
            "ident": ident.copy(),
            "ones": ones.copy(),
        })

    global _last_in_maps
    _last_in_maps = in_maps
    res = run_bass_kernel_spmd(nc, in_maps, core_ids=list(range(8)))
    out = np.empty((B, S, HID), dtype=np.float32)
    for c in range(8):
        b, g = divmod(c, G)
        out[b][:, g * CH:(g + 1) * CH] = res.results[c]["out"].T.astype(np.float32)
    return out


# revision 21
# speedup vs baseline: 1.0416x; 1.0021x over previous
"""Causal GQA self-attention on 8 Trainium2 NeuronCores.

Problem: B=2, S=2048, HIDDEN=2048, 16 q-heads, 4 kv-heads, head_dim=128, fp32 in/out.

Sharding: core c = 4*b + g  (b = batch, g = head-group).
Each core owns batch b and q-heads [4g, 4g+4) plus their shared kv-head g.

All on-chip operands are fp16 (host-cast); PSUM accumulation fp32.
Pipeline per core, interleaved so the AllGathers overlap attention compute:

  for c in 0..3:   (512-wide query chunk)
    proj(c):  QT/KT/VT for seq chunk c = W.T @ X.T chunk (output-major PSUM
              accumulation over the 16 hid k-tiles; 2 PSUM banks double-buffered)
              V[s,d] tiles via PE transpose.
    attn(c):  per head h: for key tile j<=4c+3: ST = KT_j.T @ QT_chunk,
              P = exp(ST/sqrt(d)+keybias) (ACT, fp16), causal mask mul on the
              diagonal tile (DVE), attnT += V_j.T @ P, l += ones.T @ P (PE).
              normalize attnT by 1/l -> f16 -> DMA to ag_in.
    AllGather(ag_in) across the 4 cores of the batch (runs on TOPSP/SDMA,
              overlapped with later chunks' proj+attn on the compute engines).
  for c in 0..3:   (only now; each waits only on its own AllGather)
    oproj(c): out slice [512 hid cols, chunk] = Wo_cols.T @ attnT_full(c).

Host gathers: out[b][:, 512g:512(g+1)] = core(b,g) outT.T.
"""

import numpy as np

HID = 2048
S = 2048
B = 2
NH = 16          # q heads total
D = 128          # head dim
G = 4            # head groups == cores per batch
HPG = NH // G    # q heads per group (4)
CH = 512         # seq chunk (free dim of moving operands)
NCH = S // CH    # 4 chunks
NKT = S // 128   # 16 key tiles
SCALE = 1.0 / float(np.sqrt(D))

_CACHED_NC = None


def _build_nc(sim_mode=False):
    import concourse.mybir as mybir
    import concourse.tile as tile
    from concourse import bacc

    F32 = mybir.dt.float32
    F16 = mybir.dt.float16
    Copy = mybir.ActivationFunctionType.Copy
    Exp = mybir.ActivationFunctionType.Exp

    nc = bacc.Bacc("TRN2", target_bir_lowering=False, debug=False,
                   num_devices=1 if sim_mode else 8)

    # ---- per-core input shards (fp16 except biases/keybias/out) ----
    xt = nc.declare_dram_parameter("xt", [HID, S], F16, isOutput=False)
    wq = nc.declare_dram_parameter("wq", [HID, HPG * D], F16, isOutput=False)
    wk = nc.declare_dram_parameter("wk", [HID, D], F16, isOutput=False)
    wv = nc.declare_dram_parameter("wv", [HID, D], F16, isOutput=False)
    wo = nc.declare_dram_parameter("wo", [HID, CH], F16, isOutput=False)
    cf16 = nc.declare_dram_parameter("cf16", [128, 257], F16, isOutput=False)
    cf32 = nc.declare_dram_parameter("cf32", [128, NKT + 10], F32, isOutput=False)
    out = nc.declare_dram_parameter("out", [CH, S], F16, isOutput=True)

    groups = [[0, 1, 2, 3], [4, 5, 6, 7]]

    with tile.TileContext(nc) as tc:
        with (
            tc.tile_pool(name="persist", bufs=1) as persist,
            tc.tile_pool(name="xs", bufs=2) as xs,
            tc.tile_pool(name="mo", bufs=4) as mo,
            tc.tile_pool(name="pp", bufs=6) as pp,
            tc.tile_pool(name="att", bufs=3) as att,
            tc.tile_pool(name="rbp", bufs=3) as rbp,
            tc.tile_pool(name="vts", bufs=2) as vts,
            tc.tile_pool(name="ost", bufs=4) as ost,
            tc.tile_pool(name="ps_proj", bufs=2, space="PSUM") as ps_proj_pool,
            tc.tile_pool(name="ps", bufs=3, space="PSUM") as ps_pool,
            tc.tile_pool(name="ps_pv", bufs=2, space="PSUM") as ps_pv_pool,
            tc.tile_pool(name="ps_l", bufs=1, space="PSUM") as ps_l_pool,
            tc.tile_pool(name="dram", bufs=4, space="DRAM") as dram,
        ):
            # ---- persistent SBUF state ----
            qt_sb = persist.tile([128, HPG, S], F16)       # QT per head [d, h, s]
            kt_sb = persist.tile([128, S], F16)            # KT          [d, s]
            v_sb = persist.tile([128, NKT, D], F16)        # V           [s, j, d]
            wq_sb = persist.tile([128, NKT, HPG * D], F16)
            wk_sb = persist.tile([128, NKT, D], F16)
            wv_sb = persist.tile([128, NKT, D], F16)
            wo_sb = persist.tile([128, NKT, CH], F16)
            cf16_sb = persist.tile([128, 257], F16)
            cf32_sb = persist.tile([128, NKT + 10], F32)
            nc.sync.dma_start(out=cf16_sb, in_=cf16.ap())
            nc.scalar.dma_start(out=cf32_sb, in_=cf32.ap())
            mask_sb = cf16_sb[:, 0:128]
            idr_sb = cf16_sb[:, 128:256]
            ones_sb = cf16_sb[:, 256:257]
            kb_sb = cf32_sb[:, 0:NKT]
            bq_sb = cf32_sb[:, NKT:NKT + 4]
            bk_sb = cf32_sb[:, NKT + 4:NKT + 5]
            bv_sb = cf32_sb[:, NKT + 5:NKT + 6]
            bo_sb = cf32_sb[:, NKT + 6:NKT + 10]

            # tiny warmup collective: absorbs rank launch skew + first-call
            # ncfw setup while the projections run, so the first real
            # AllGather runs at steady-state latency
            warm_in = dram.tile([128, 8], F16, name="warm_in")
            warm_out = dram.tile([G, 128, 8], F16, name="warm_out")
            nc.sync.dma_start(out=warm_in[:, 0:1], in_=ones_sb)
            if not sim_mode:
                nc.gpsimd.collective_compute(
                    "AllGather", mybir.AluOpType.bypass,
                    replica_groups=groups,
                    ins=[warm_in.opt()], outs=[warm_out.opt()],
                )

            # weights: (t p) n -> p t n so partition dim is hid-within-tile.
            # xt chunk 0 + per-k-tile weight transfers issue first and spread
            # across DMA queues so the first matmuls start within a few us.
            wq_r = wq.ap().rearrange("(t p) n -> p t n", p=128)
            wk_r = wk.ap().rearrange("(t p) n -> p t n", p=128)
            wv_r = wv.ap().rearrange("(t p) n -> p t n", p=128)
            wo_r = wo.ap().rearrange("(t p) n -> p t n", p=128)

            xt_r = xt.ap().rearrange("(t p) n -> p t n", p=128)

            def load_xt(c):
                xt_c = xs.tile([128, NKT, CH], F16, name="xt_c")
                sq = slice(c * CH, (c + 1) * CH)
                engs = [nc.sync, nc.scalar, nc.gpsimd, nc.sync]
                for u in range(4):
                    engs[u].dma_start(out=xt_c[:, 4 * u:4 * (u + 1), :],
                                      in_=xt_r[:, 4 * u:4 * (u + 1), sq])
                return xt_c

            xt_tiles = {0: load_xt(0)}
            for u in range(4):
                nc.gpsimd.dma_start(out=wq_sb[:, 4 * u:4 * (u + 1), :],
                                    in_=wq_r[:, 4 * u:4 * (u + 1), :])
            nc.gpsimd.dma_start(out=wk_sb, in_=wk_r)
            nc.gpsimd.dma_start(out=wv_sb, in_=wv_r)

            m_alls = []
            out_r = out.ap().rearrange("(t p) n -> p t n", p=128)

            def oproj(c):
                sq = slice(c * CH, (c + 1) * CH)
                m_all = m_alls[c]
                for t in range(HPG):
                    ps_o = ps_pool.tile([128, CH], F32, name="ps_any")
                    for ct in range(NKT):
                        nc.tensor.matmul(
                            ps_o, lhsT=wo_sb[:, ct, t * 128:(t + 1) * 128],
                            rhs=m_all[:, ct, :],
                            start=(ct == 0), stop=(ct == NKT - 1))
                    o_sb = ost.tile([128, CH], F16, name="o_sb")
                    nc.vector.tensor_scalar_add(o_sb, ps_o, bo_sb[:, t:t + 1])
                    eng = nc.sync if t % 2 == 0 else nc.gpsimd
                    eng.dma_start(out=out_r[:, t, sq], in_=o_sb)

            for c in range(NCH):
                sq = slice(c * CH, (c + 1) * CH)

                # ================= projections for chunk c =================
                # prefetch next chunk's xt ahead of this chunk's m_all loads
                if c + 1 < NCH:
                    xt_tiles[c + 1] = load_xt(c + 1)
                if c == 1:
                    nc.scalar.dma_start(out=wo_sb[:, :8, :], in_=wo_r[:, :8, :])
                    nc.scalar.dma_start(out=wo_sb[:, 8:, :], in_=wo_r[:, 8:, :])
                xt_c = xt_tiles[c]
                for h in range(HPG):
                    ps_q = ps_proj_pool.tile([128, CH], F32, name="ps_proj")
                    for t in range(NKT):
                        nc.tensor.matmul(ps_q,
                                         lhsT=wq_sb[:, t, h * D:(h + 1) * D],
                                         rhs=xt_c[:, t, :],
                                         start=(t == 0), stop=(t == NKT - 1))
                    nc.vector.tensor_scalar_add(qt_sb[:, h, sq], ps_q,
                                                bq_sb[:, h:h + 1])
                ps_k = ps_proj_pool.tile([128, CH], F32, name="ps_proj")
                for t in range(NKT):
                    nc.tensor.matmul(ps_k, lhsT=wk_sb[:, t, :], rhs=xt_c[:, t, :],
                                     start=(t == 0), stop=(t == NKT - 1))
                nc.vector.tensor_scalar_add(kt_sb[:, sq], ps_k, bk_sb)
                ps_v = ps_proj_pool.tile([128, CH], F32, name="ps_proj")
                for t in range(NKT):
                    nc.tensor.matmul(ps_v, lhsT=wv_sb[:, t, :], rhs=xt_c[:, t, :],
                                     start=(t == 0), stop=(t == NKT - 1))
                vt_t = vts.tile([128, CH], F16, name="vt_t")
                nc.vector.tensor_scalar_add(vt_t, ps_v, bv_sb)
                for u in range(4):
                    j = 4 * c + u
                    ps_tp = ps_pool.tile([128, 128], F16, name="ps_any")
                    nc.tensor.transpose(
                        ps_tp, vt_t[:, u * 128:(u + 1) * 128], idr_sb)
                    nc.scalar.activation(v_sb[:, j, :], ps_tp, Copy)

                # ================= attention for chunk c =================
                # heads processed in pairs: shared KT_j/V_j stationary loads,
                # rowsums packed into one PE pass via col-tiling (M=1 each at
                # col-groups 0 and 32), one reciprocal per pair.
                njt = 4 * c + 4  # causal: key tiles 0..4c+3
                ag_in = dram.tile([HPG, 128, CH], F16, name="ag_in")
                for pr in range(HPG // 2):
                    ha, hb = 2 * pr, 2 * pr + 1
                    ps_pv_a = ps_pv_pool.tile([128, CH], F32, name="ps_pv")
                    ps_pv_b = ps_pv_pool.tile([128, CH], F32, name="ps_pv")
                    ps_l = ps_l_pool.tile([33, CH], F32, name="ps_l")
                    for j in range(njt):
                        # key tile j only reaches queries >= 128*r into the
                        # chunk (r = j - 4c >= 0 on the diagonal)
                        r = max(0, j - 4 * c)
                        cs = slice(128 * r, CH)
                        qs_ = slice(c * CH + 128 * r, (c + 1) * CH)
                        kjt = kt_sb[:, j * 128:(j + 1) * 128]
                        ps_s_a = ps_pool.tile([128, CH], F32, name="ps_any")
                        ps_s_b = ps_pool.tile([128, CH], F32, name="ps_any")
                        nc.tensor.matmul(ps_s_a[:, cs], lhsT=kjt,
                                         rhs=qt_sb[:, ha, qs_],
                                         start=True, stop=True)
                        nc.tensor.matmul(ps_s_b[:, cs], lhsT=kjt,
                                         rhs=qt_sb[:, hb, qs_],
                                         start=True, stop=True)
                        p_a = pp.tile([128, CH], F16, name="p_a")
                        p_b = pp.tile([128, CH], F16, name="p_b")
                        nc.scalar.activation(p_a[:, cs], ps_s_a[:, cs], Exp,
                                             scale=SCALE, bias=kb_sb[:, j:j + 1])
                        nc.scalar.activation(p_b[:, cs], ps_s_b[:, cs], Exp,
                                             scale=SCALE, bias=kb_sb[:, j:j + 1])
                        if j >= 4 * c:
                            # triangular mask on the 128-wide diagonal block
                            ds = slice(128 * r, 128 * (r + 1))
                            nc.vector.tensor_mul(p_a[:, ds], p_a[:, ds], mask_sb)
                            nc.vector.tensor_mul(p_b[:, ds], p_b[:, ds], mask_sb)
                        st, sp = (j == 0), (j == njt - 1)
                        nc.tensor.matmul(ps_pv_a[:, cs], lhsT=v_sb[:, j, :],
                                         rhs=p_a[:, cs], start=st, stop=sp)
                        nc.tensor.matmul(ps_pv_b[:, cs], lhsT=v_sb[:, j, :],
                                         rhs=p_b[:, cs], start=st, stop=sp)
                        # two interleaved rowsum accumulation groups share one
                        # PSUM bank (partitions 0 / 32); col-tiled so the two
                        # M=1 matmuls run concurrently in the PE array.
                        nc.tensor.matmul(ps_l[0:1, cs], lhsT=ones_sb,
                                         rhs=p_a[:, cs], start=st, stop=sp,
                                         tile_position=(0, 0),
                                         skip_group_check=True)
                        nc.tensor.matmul(ps_l[32:33, cs], lhsT=ones_sb,
                                         rhs=p_b[:, cs], start=st, stop=sp,
                                         tile_position=(0, 32),
                                         skip_group_check=True)
                    # partition_broadcast requires its input at partition 0,
                    # so each reciprocal lands in its own partition-0 tile
                    rl_a = rbp.tile([1, CH], F32, name="rl_a")
                    rl_b = rbp.tile([1, CH], F32, name="rl_b")
                    nc.vector.reciprocal(rl_a, ps_l[0:1, :])
                    nc.vector.reciprocal(rl_b, ps_l[32:33, :])
                    rb_a = rbp.tile([128, CH], F32, name="rb_a")
                    rb_b = rbp.tile([128, CH], F32, name="rb_b")
                    nc.gpsimd.partition_broadcast(rb_a, rl_a, channels=128)
                    nc.gpsimd.partition_broadcast(rb_b, rl_b, channels=128)
                    at_a = att.tile([128, CH], F16, name="at_sb")
                    at_b = att.tile([128, CH], F16, name="at_sb")
                    nc.vector.tensor_mul(at_a, ps_pv_a, rb_a)
                    nc.vector.tensor_mul(at_b, ps_pv_b, rb_b)
                    nc.sync.dma_start(out=ag_in[ha], in_=at_a)
                    nc.sync.dma_start(out=ag_in[hb], in_=at_b)

                ag_out = dram.tile([G, HPG, 128, CH], F16, name="ag_out")
                if sim_mode:
                    # stand-in for the AllGather with equivalent local IO
                    for g in range(G):
                        nc.sync.dma_start(out=ag_out[g], in_=ag_in[:])
                else:
                    nc.gpsimd.collective_compute(
                        "AllGather", mybir.AluOpType.bypass,
                        replica_groups=groups,
                        ins=[ag_in.opt()], outs=[ag_out.opt()],
                    )
                m_all = mo.tile([128, NKT, CH], F16, name="m_all")
                ag_r = ag_out.rearrange("g h p n -> p (g h) n")
                nc.scalar.dma_start(out=m_all[:, :8, :], in_=ag_r[:, :8, :])
                nc.gpsimd.dma_start(out=m_all[:, 8:, :], in_=ag_r[:, 8:, :])
                m_alls.append(m_all)

            # ================= O-projections =================
            for c in range(NCH):
                oproj(c)

    nc.compile()
    return nc


def _host_consts():
    # f16 blob: [triangular mask | identity | ones]; mask[p, col] = col >= p
    col = np.arange(128)[None, :]
    p = np.arange(128)[:, None]
    maskt = (col >= p).astype(np.float16)
    ident = np.eye(128, dtype=np.float16)
    ones = np.ones((128, 1), dtype=np.float16)
    return np.ascontiguousarray(np.concatenate([maskt, ident, ones], axis=1))


def kernel(hidden_states, attention_mask, Wq, bq, Wk, bk, Wv, bv, Wo, bo):
    from concourse.bass_utils import run_bass_kernel_spmd

    global _CACHED_NC
    if _CACHED_NC is None:
        _CACHED_NC = _build_nc()
    nc = _CACHED_NC

    X = np.asarray(hidden_states, dtype=np.float32).astype(np.float16)
    am = np.asarray(attention_mask).astype(np.float32)  # [B, S] key mask
    Wq16 = np.asarray(Wq, np.float32).astype(np.float16)
    Wk16 = np.asarray(Wk, np.float32).astype(np.float16)
    Wv16 = np.asarray(Wv, np.float32).astype(np.float16)
    Wo16 = np.asarray(Wo, np.float32).astype(np.float16)
    cf16_host = _host_consts()

    in_maps = []
    for c in range(8):
        b, g = divmod(c, G)
        qs = slice(g * HPG * D, (g + 1) * HPG * D)   # q-head cols of group g
        ks = slice(g * D, (g + 1) * D)               # kv-head cols of group g
        in_maps.append({
            "xt": np.ascontiguousarray(X[b].T),
            "wq": np.ascontiguousarray(Wq16[:, qs]),
            "wk": np.ascontiguousarray(Wk16[:, ks]),
            "wv": np.ascontiguousarray(Wv16[:, ks]),
            "wo": np.ascontiguousarray(Wo16[:, qs]),  # hid cols [512g, 512g+512)
            "cf16": cf16_host.copy(),
            "cf32": np.ascontiguousarray(np.concatenate([
                ((1.0 - am[b]) * -10000.0).astype(np.float32)
                .reshape(NKT, 128).T,
                np.asarray(bq, np.float32)[qs].reshape(HPG, D).T,
                np.asarray(bk, np.float32)[ks].reshape(D, 1),
                np.asarray(bv, np.float32)[ks].reshape(D, 1),
                np.asarray(bo, np.float32)[qs].reshape(HPG, D).T,
            ], axis=1)),
        })

    global _last_in_maps
    _last_in_maps = in_maps
    res = run_bass_kernel_spmd(nc, in_maps, core_ids=list(range(8)))
    out = np.empty((B, S, HID), dtype=np.float32)
    for c in range(8):
        b, g = divmod(c, G)
        out[b][:, g * CH:(g + 1) * CH] = res.results[c]["out"].T.astype(np.float32)
    return out


# revision 22
# speedup vs baseline: 1.0628x; 1.0203x over previous
"""Causal GQA self-attention on 8 Trainium2 NeuronCores.

Problem: B=2, S=2048, HIDDEN=2048, 16 q-heads, 4 kv-heads, head_dim=128, fp32 in/out.

Sharding: core c = 4*b + g  (b = batch, g = head-group).
Each core owns batch b and q-heads [4g, 4g+4) plus their shared kv-head g.

All on-chip operands are fp16 (host-cast); PSUM accumulation fp32.
Pipeline per core, interleaved so the AllGathers overlap attention compute:

  for c in 0..3:   (512-wide query chunk)
    proj(c):  QT/KT/VT for seq chunk c = W.T @ X.T chunk (output-major PSUM
              accumulation over the 16 hid k-tiles; 2 PSUM banks double-buffered)
              V[s,d] tiles via PE transpose.
    attn(c):  per head h: for key tile j<=4c+3: ST = KT_j.T @ QT_chunk,
              P = exp(ST/sqrt(d)+keybias) (ACT, fp16), causal mask mul on the
              diagonal tile (DVE), attnT += V_j.T @ P, l += ones.T @ P (PE).
              normalize attnT by 1/l -> f16 -> DMA to ag_in.
    AllGather(ag_in) across the 4 cores of the batch (runs on TOPSP/SDMA,
              overlapped with later chunks' proj+attn on the compute engines).
  for c in 0..3:   (only now; each waits only on its own AllGather)
    oproj(c): out slice [512 hid cols, chunk] = Wo_cols.T @ attnT_full(c).

Host gathers: out[b][:, 512g:512(g+1)] = core(b,g) outT.T.
"""

import numpy as np

HID = 2048
S = 2048
B = 2
NH = 16          # q heads total
D = 128          # head dim
G = 4            # head groups == cores per batch
HPG = NH // G    # q heads per group (4)
CH = 512         # seq chunk (free dim of moving operands)
NCH = S // CH    # 4 chunks
NKT = S // 128   # 16 key tiles
SCALE = 1.0 / float(np.sqrt(D))

_CACHED_NC = None


def _build_nc(sim_mode=False):
    import concourse.mybir as mybir
    import concourse.tile as tile
    from concourse import bacc

    F32 = mybir.dt.float32
    F16 = mybir.dt.float16
    Copy = mybir.ActivationFunctionType.Copy
    Exp = mybir.ActivationFunctionType.Exp

    nc = bacc.Bacc("TRN2", target_bir_lowering=False, debug=False,
                   num_devices=1 if sim_mode else 8)

    # ---- per-core input shards (fp16 except biases/keybias/out) ----
    xt = nc.declare_dram_parameter("xt", [HID, S], F16, isOutput=False)
    wq = nc.declare_dram_parameter("wq", [HID, HPG * D], F16, isOutput=False)
    wk = nc.declare_dram_parameter("wk", [HID, D], F16, isOutput=False)
    wv = nc.declare_dram_parameter("wv", [HID, D], F16, isOutput=False)
    wo = nc.declare_dram_parameter("wo", [HID, CH], F16, isOutput=False)
    cf16 = nc.declare_dram_parameter("cf16", [128, 257], F16, isOutput=False)
    cf32 = nc.declare_dram_parameter("cf32", [128, NKT + 10], F32, isOutput=False)
    out = nc.declare_dram_parameter("out", [CH, S], F16, isOutput=True)

    groups = [[0, 1, 2, 3], [4, 5, 6, 7]]

    with tile.TileContext(nc) as tc:
        with (
            tc.tile_pool(name="persist", bufs=1) as persist,
            tc.tile_pool(name="xs", bufs=2) as xs,
            tc.tile_pool(name="mo", bufs=4) as mo,
            tc.tile_pool(name="pp", bufs=6) as pp,
            tc.tile_pool(name="att", bufs=3) as att,
            tc.tile_pool(name="rbp", bufs=3) as rbp,
            tc.tile_pool(name="vts", bufs=2) as vts,
            tc.tile_pool(name="ost", bufs=4) as ost,
            tc.tile_pool(name="ps_proj", bufs=2, space="PSUM") as ps_proj_pool,
            tc.tile_pool(name="ps", bufs=3, space="PSUM") as ps_pool,
            tc.tile_pool(name="ps_pv", bufs=2, space="PSUM") as ps_pv_pool,
            tc.tile_pool(name="ps_l", bufs=1, space="PSUM") as ps_l_pool,
            tc.tile_pool(name="dram", bufs=4, space="DRAM") as dram,
        ):
            # ---- persistent SBUF state ----
            qt_sb = persist.tile([128, HPG, S], F16)       # QT per head [d, h, s]
            kt_sb = persist.tile([128, S], F16)            # KT          [d, s]
            v_sb = persist.tile([128, NKT, D], F16)        # V           [s, j, d]
            wq_sb = persist.tile([128, NKT, HPG * D], F16)
            wk_sb = persist.tile([128, NKT, D], F16)
            wv_sb = persist.tile([128, NKT, D], F16)
            wo_sb = persist.tile([128, NKT, CH], F16)
            cf16_sb = persist.tile([128, 257], F16)
            cf32_sb = persist.tile([128, NKT + 10], F32)
            nc.sync.dma_start(out=cf16_sb, in_=cf16.ap())
            nc.scalar.dma_start(out=cf32_sb, in_=cf32.ap())
            mask_sb = cf16_sb[:, 0:128]
            idr_sb = cf16_sb[:, 128:256]
            ones_sb = cf16_sb[:, 256:257]
            kb_sb = cf32_sb[:, 0:NKT]
            bq_sb = cf32_sb[:, NKT:NKT + 4]
            bk_sb = cf32_sb[:, NKT + 4:NKT + 5]
            bv_sb = cf32_sb[:, NKT + 5:NKT + 6]
            bo_sb = cf32_sb[:, NKT + 6:NKT + 10]

            # tiny warmup collective: absorbs rank launch skew + first-call
            # ncfw setup while the projections run, so the first real
            # AllGather runs at steady-state latency
            warm_in = dram.tile([128, 8], F16, name="warm_in")
            warm_out = dram.tile([G, 128, 8], F16, name="warm_out")
            nc.sync.dma_start(out=warm_in[:, 0:1], in_=ones_sb)
            if not sim_mode:
                nc.gpsimd.collective_compute(
                    "AllGather", mybir.AluOpType.bypass,
                    replica_groups=groups,
                    ins=[warm_in.opt()], outs=[warm_out.opt()],
                )

            # weights: (t p) n -> p t n so partition dim is hid-within-tile.
            # xt chunk 0 + per-k-tile weight transfers issue first and spread
            # across DMA queues so the first matmuls start within a few us.
            wq_r = wq.ap().rearrange("(t p) n -> p t n", p=128)
            wk_r = wk.ap().rearrange("(t p) n -> p t n", p=128)
            wv_r = wv.ap().rearrange("(t p) n -> p t n", p=128)
            wo_r = wo.ap().rearrange("(t p) n -> p t n", p=128)

            xt_r = xt.ap().rearrange("(t p) n -> p t n", p=128)

            def load_xt(c):
                xt_c = xs.tile([128, NKT, CH], F16, name="xt_c")
                sq = slice(c * CH, (c + 1) * CH)
                engs = [nc.sync, nc.sync, nc.sync, nc.sync]
                for u in range(4):
                    engs[u].dma_start(out=xt_c[:, 4 * u:4 * (u + 1), :],
                                      in_=xt_r[:, 4 * u:4 * (u + 1), sq])
                return xt_c

            xt_tiles = {0: load_xt(0)}
            for u in range(4):
                nc.gpsimd.dma_start(out=wq_sb[:, 4 * u:4 * (u + 1), :],
                                    in_=wq_r[:, 4 * u:4 * (u + 1), :])
            nc.gpsimd.dma_start(out=wk_sb, in_=wk_r)
            nc.gpsimd.dma_start(out=wv_sb, in_=wv_r)

            m_alls = []
            out_r = out.ap().rearrange("(t p) n -> p t n", p=128)

            def oproj(c):
                sq = slice(c * CH, (c + 1) * CH)
                m_all = m_alls[c]
                for t in range(HPG):
                    ps_o = ps_pool.tile([128, CH], F32, name="ps_any")
                    for ct in range(NKT):
                        nc.tensor.matmul(
                            ps_o, lhsT=wo_sb[:, ct, t * 128:(t + 1) * 128],
                            rhs=m_all[:, ct, :],
                            start=(ct == 0), stop=(ct == NKT - 1))
                    o_sb = ost.tile([128, CH], F16, name="o_sb")
                    nc.vector.tensor_scalar_add(o_sb, ps_o, bo_sb[:, t:t + 1])
                    eng = nc.sync if t % 2 == 0 else nc.gpsimd
                    eng.dma_start(out=out_r[:, t, sq], in_=o_sb)

            for c in range(NCH):
                sq = slice(c * CH, (c + 1) * CH)

                # ================= projections for chunk c =================
                # prefetch next chunk's xt ahead of this chunk's m_all loads
                if c + 1 < NCH:
                    xt_tiles[c + 1] = load_xt(c + 1)
                if c == 1:
                    nc.scalar.dma_start(out=wo_sb[:, :8, :], in_=wo_r[:, :8, :])
                    nc.scalar.dma_start(out=wo_sb[:, 8:, :], in_=wo_r[:, 8:, :])
                xt_c = xt_tiles[c]
                for h in range(HPG):
                    ps_q = ps_proj_pool.tile([128, CH], F32, name="ps_proj")
                    for t in range(NKT):
                        nc.tensor.matmul(ps_q,
                                         lhsT=wq_sb[:, t, h * D:(h + 1) * D],
                                         rhs=xt_c[:, t, :],
                                         start=(t == 0), stop=(t == NKT - 1))
                    nc.vector.tensor_scalar_add(qt_sb[:, h, sq], ps_q,
                                                bq_sb[:, h:h + 1])
                ps_k = ps_proj_pool.tile([128, CH], F32, name="ps_proj")
                for t in range(NKT):
                    nc.tensor.matmul(ps_k, lhsT=wk_sb[:, t, :], rhs=xt_c[:, t, :],
                                     start=(t == 0), stop=(t == NKT - 1))
                nc.vector.tensor_scalar_add(kt_sb[:, sq], ps_k, bk_sb)
                ps_v = ps_proj_pool.tile([128, CH], F32, name="ps_proj")
                for t in range(NKT):
                    nc.tensor.matmul(ps_v, lhsT=wv_sb[:, t, :], rhs=xt_c[:, t, :],
                                     start=(t == 0), stop=(t == NKT - 1))
                vt_t = vts.tile([128, CH], F16, name="vt_t")
                nc.vector.tensor_scalar_add(vt_t, ps_v, bv_sb)
                for u in range(4):
                    j = 4 * c + u
                    ps_tp = ps_pool.tile([128, 128], F16, name="ps_any")
                    nc.tensor.transpose(
                        ps_tp, vt_t[:, u * 128:(u + 1) * 128], idr_sb)
                    nc.scalar.activation(v_sb[:, j, :], ps_tp, Copy)

                # ================= attention for chunk c =================
                if c == NCH - 1:
                    m_all_last = mo.tile([128, NKT, CH], F16, name="m_all")
                # heads processed in pairs: shared KT_j/V_j stationary loads,
                # rowsums packed into one PE pass via col-tiling (M=1 each at
                # col-groups 0 and 32), one reciprocal per pair.
                njt = 4 * c + 4  # causal: key tiles 0..4c+3
                if c == NCH - 1:
                    ag_pair = [dram.tile([2, 128, CH], F16, name="ag_pa"),
                               dram.tile([2, 128, CH], F16, name="ag_pb")]
                else:
                    ag_in = dram.tile([HPG, 128, CH], F16, name="ag_in")
                for pr in range(HPG // 2):
                    ha, hb = 2 * pr, 2 * pr + 1
                    ps_pv_a = ps_pv_pool.tile([128, CH], F32, name="ps_pv")
                    ps_pv_b = ps_pv_pool.tile([128, CH], F32, name="ps_pv")
                    ps_l = ps_l_pool.tile([33, CH], F32, name="ps_l")
                    for j in range(njt):
                        # key tile j only reaches queries >= 128*r into the
                        # chunk (r = j - 4c >= 0 on the diagonal)
                        r = max(0, j - 4 * c)
                        cs = slice(128 * r, CH)
                        qs_ = slice(c * CH + 128 * r, (c + 1) * CH)
                        kjt = kt_sb[:, j * 128:(j + 1) * 128]
                        ps_s_a = ps_pool.tile([128, CH], F32, name="ps_any")
                        ps_s_b = ps_pool.tile([128, CH], F32, name="ps_any")
                        nc.tensor.matmul(ps_s_a[:, cs], lhsT=kjt,
                                         rhs=qt_sb[:, ha, qs_],
                                         start=True, stop=True)
                        nc.tensor.matmul(ps_s_b[:, cs], lhsT=kjt,
                                         rhs=qt_sb[:, hb, qs_],
                                         start=True, stop=True)
                        p_a = pp.tile([128, CH], F16, name="p_a")
                        p_b = pp.tile([128, CH], F16, name="p_b")
                        nc.scalar.activation(p_a[:, cs], ps_s_a[:, cs], Exp,
                                             scale=SCALE, bias=kb_sb[:, j:j + 1])
                        nc.scalar.activation(p_b[:, cs], ps_s_b[:, cs], Exp,
                                             scale=SCALE, bias=kb_sb[:, j:j + 1])
                        if j >= 4 * c:
                            # triangular mask on the 128-wide diagonal block
                            ds = slice(128 * r, 128 * (r + 1))
                            nc.vector.tensor_mul(p_a[:, ds], p_a[:, ds], mask_sb)
                            nc.vector.tensor_mul(p_b[:, ds], p_b[:, ds], mask_sb)
                        st, sp = (j == 0), (j == njt - 1)
                        nc.tensor.matmul(ps_pv_a[:, cs], lhsT=v_sb[:, j, :],
                                         rhs=p_a[:, cs], start=st, stop=sp)
                        nc.tensor.matmul(ps_pv_b[:, cs], lhsT=v_sb[:, j, :],
                                         rhs=p_b[:, cs], start=st, stop=sp)
                        # two interleaved rowsum accumulation groups share one
                        # PSUM bank (partitions 0 / 32); col-tiled so the two
                        # M=1 matmuls run concurrently in the PE array.
                        nc.tensor.matmul(ps_l[0:1, cs], lhsT=ones_sb,
                                         rhs=p_a[:, cs], start=st, stop=sp,
                                         tile_position=(0, 0),
                                         skip_group_check=True)
                        nc.tensor.matmul(ps_l[32:33, cs], lhsT=ones_sb,
                                         rhs=p_b[:, cs], start=st, stop=sp,
                                         tile_position=(0, 32),
                                         skip_group_check=True)
                    # partition_broadcast requires its input at partition 0,
                    # so each reciprocal lands in its own partition-0 tile
                    rl_a = rbp.tile([1, CH], F32, name="rl_a")
                    rl_b = rbp.tile([1, CH], F32, name="rl_b")
                    nc.vector.reciprocal(rl_a, ps_l[0:1, :])
                    nc.vector.reciprocal(rl_b, ps_l[32:33, :])
                    rb_a = rbp.tile([128, CH], F32, name="rb_a")
                    rb_b = rbp.tile([128, CH], F32, name="rb_b")
                    nc.gpsimd.partition_broadcast(rb_a, rl_a, channels=128)
                    nc.gpsimd.partition_broadcast(rb_b, rl_b, channels=128)
                    at_a = att.tile([128, CH], F16, name="at_sb")
                    at_b = att.tile([128, CH], F16, name="at_sb")
                    nc.vector.tensor_mul(at_a, ps_pv_a, rb_a)
                    nc.vector.tensor_mul(at_b, ps_pv_b, rb_b)
                    if c == NCH - 1:
                        nc.scalar.dma_start(out=ag_pair[pr][0], in_=at_a)
                        nc.scalar.dma_start(out=ag_pair[pr][1], in_=at_b)
                        ago = dram.tile([G, 2, 128, CH], F16, name="ag_po")
                        if sim_mode:
                            for g in range(G):
                                nc.sync.dma_start(out=ago[g], in_=ag_pair[pr][:])
                        else:
                            nc.gpsimd.collective_compute(
                                "AllGather", mybir.AluOpType.bypass,
                                replica_groups=groups,
                                ins=[ag_pair[pr].opt()], outs=[ago.opt()],
                            )
                        agr = ago.rearrange("g h p n -> p (g h) n")
                        m3 = m_all_last
                        for g in range(G):
                            eng = nc.scalar if g % 2 == 0 else nc.gpsimd
                            eng.dma_start(
                                out=m3[:, 4 * g + 2 * pr:4 * g + 2 * pr + 2, :],
                                in_=agr[:, 2 * g:2 * g + 2, :])
                    else:
                        nc.scalar.dma_start(out=ag_in[ha], in_=at_a)
                        nc.scalar.dma_start(out=ag_in[hb], in_=at_b)

                if c == NCH - 1:
                    m_alls.append(m_all_last)
                else:
                    ag_out = dram.tile([G, HPG, 128, CH], F16, name="ag_out")
                    if sim_mode:
                        # stand-in for the AllGather with equivalent local IO
                        for g in range(G):
                            nc.sync.dma_start(out=ag_out[g], in_=ag_in[:])
                    else:
                        nc.gpsimd.collective_compute(
                            "AllGather", mybir.AluOpType.bypass,
                            replica_groups=groups,
                            ins=[ag_in.opt()], outs=[ag_out.opt()],
                        )
                    m_all = mo.tile([128, NKT, CH], F16, name="m_all")
                    ag_r = ag_out.rearrange("g h p n -> p (g h) n")
                    nc.scalar.dma_start(out=m_all[:, :8, :], in_=ag_r[:, :8, :])
                    nc.gpsimd.dma_start(out=m_all[:, 8:, :], in_=ag_r[:, 8:, :])
                    m_alls.append(m_all)

            # ================= O-projections =================
            for c in range(NCH):
                oproj(c)

    nc.compile()
    return nc


def _host_consts():
    # f16 blob: [triangular mask | identity | ones]; mask[p, col] = col >= p
    col = np.arange(128)[None, :]
    p = np.arange(128)[:, None]
    maskt = (col >= p).astype(np.float16)
    ident = np.eye(128, dtype=np.float16)
    ones = np.ones((128, 1), dtype=np.float16)
    return np.ascontiguousarray(np.concatenate([maskt, ident, ones], axis=1))


def kernel(hidden_states, attention_mask, Wq, bq, Wk, bk, Wv, bv, Wo, bo):
    from concourse.bass_utils import run_bass_kernel_spmd

    global _CACHED_NC
    if _CACHED_NC is None:
        _CACHED_NC = _build_nc()
    nc = _CACHED_NC

    X = np.asarray(hidden_states, dtype=np.float32).astype(np.float16)
    am = np.asarray(attention_mask).astype(np.float32)  # [B, S] key mask
    Wq16 = np.asarray(Wq, np.float32).astype(np.float16)
    Wk16 = np.asarray(Wk, np.float32).astype(np.float16)
    Wv16 = np.asarray(Wv, np.float32).astype(np.float16)
    Wo16 = np.asarray(Wo, np.float32).astype(np.float16)
    cf16_host = _host_consts()

    in_maps = []
    for c in range(8):
        b, g = divmod(c, G)
        qs = slice(g * HPG * D, (g + 1) * HPG * D)   # q-head cols of group g
        ks = slice(g * D, (g + 1) * D)               # kv-head cols of group g
        in_maps.append({
            "xt": np.ascontiguousarray(X[b].T),
            "wq": np.ascontiguousarray(Wq16[:, qs]),
            "wk": np.ascontiguousarray(Wk16[:, ks]),
            "wv": np.ascontiguousarray(Wv16[:, ks]),
            "wo": np.ascontiguousarray(Wo16[:, qs]),  # hid cols [512g, 512g+512)
            "cf16": cf16_host.copy(),
            "cf32": np.ascontiguousarray(np.concatenate([
                ((1.0 - am[b]) * -10000.0).astype(np.float32)
                .reshape(NKT, 128).T,
                np.asarray(bq, np.float32)[qs].reshape(HPG, D).T,
                np.asarray(bk, np.float32)[ks].reshape(D, 1),
                np.asarray(bv, np.float32)[ks].reshape(D, 1),
                np.asarray(bo, np.float32)[qs].reshape(HPG, D).T,
            ], axis=1)),
        })

    global _last_in_maps
    _last_in_maps = in_maps
    res = run_bass_kernel_spmd(nc, in_maps, core_ids=list(range(8)))
    out = np.empty((B, S, HID), dtype=np.float32)
    for c in range(8):
        b, g = divmod(c, G)
        out[b][:, g * CH:(g + 1) * CH] = res.results[c]["out"].T.astype(np.float32)
    return out


# revision 23
# speedup vs baseline: 1.0738x; 1.0104x over previous
"""Causal GQA self-attention on 8 Trainium2 NeuronCores.

Problem: B=2, S=2048, HIDDEN=2048, 16 q-heads, 4 kv-heads, head_dim=128, fp32 in/out.

Sharding: core c = 4*b + g  (b = batch, g = head-group).
Each core owns batch b and q-heads [4g, 4g+4) plus their shared kv-head g.

All on-chip operands are fp16 (host-cast); PSUM accumulation fp32.
Pipeline per core, interleaved so the AllGathers overlap attention compute:

  for c in 0..3:   (512-wide query chunk)
    proj(c):  QT/KT/VT for seq chunk c = W.T @ X.T chunk (output-major PSUM
              accumulation over the 16 hid k-tiles; 2 PSUM banks double-buffered)
              V[s,d] tiles via PE transpose.
    attn(c):  per head h: for key tile j<=4c+3: ST = KT_j.T @ QT_chunk,
              P = exp(ST/sqrt(d)+keybias) (ACT, fp16), causal mask mul on the
              diagonal tile (DVE), attnT += V_j.T @ P, l += ones.T @ P (PE).
              normalize attnT by 1/l -> f16 -> DMA to ag_in.
    AllGather(ag_in) across the 4 cores of the batch (runs on TOPSP/SDMA,
              overlapped with later chunks' proj+attn on the compute engines).
  for c in 0..3:   (only now; each waits only on its own AllGather)
    oproj(c): out slice [512 hid cols, chunk] = Wo_cols.T @ attnT_full(c).

Host gathers: out[b][:, 512g:512(g+1)] = core(b,g) outT.T.
"""

import numpy as np

HID = 2048
S = 2048
B = 2
NH = 16          # q heads total
D = 128          # head dim
G = 4            # head groups == cores per batch
HPG = NH // G    # q heads per group (4)
CH = 512         # seq chunk (free dim of moving operands)
NCH = S // CH    # 4 chunks
NKT = S // 128   # 16 key tiles
SCALE = 1.0 / float(np.sqrt(D))

_CACHED_NC = None


def _build_nc(sim_mode=False):
    import concourse.mybir as mybir
    import concourse.tile as tile
    from concourse import bacc

    F32 = mybir.dt.float32
    F16 = mybir.dt.float16
    Copy = mybir.ActivationFunctionType.Copy
    Exp = mybir.ActivationFunctionType.Exp

    nc = bacc.Bacc("TRN2", target_bir_lowering=False, debug=False,
                   num_devices=1 if sim_mode else 8)

    # ---- per-core input shards (fp16 except biases/keybias/out) ----
    xt = nc.declare_dram_parameter("xt", [HID, S], F16, isOutput=False)
    wq = nc.declare_dram_parameter("wq", [HID, HPG * D], F16, isOutput=False)
    wk = nc.declare_dram_parameter("wk", [HID, D], F16, isOutput=False)
    wv = nc.declare_dram_parameter("wv", [HID, D], F16, isOutput=False)
    wo = nc.declare_dram_parameter("wo", [HID, CH], F16, isOutput=False)
    cf16 = nc.declare_dram_parameter("cf16", [128, 257], F16, isOutput=False)
    cf32 = nc.declare_dram_parameter("cf32", [128, NKT + 10], F32, isOutput=False)
    out = nc.declare_dram_parameter("out", [CH, S], F16, isOutput=True)

    groups = [[0, 1, 2, 3], [4, 5, 6, 7]]

    with tile.TileContext(nc) as tc:
        with (
            tc.tile_pool(name="persist", bufs=1) as persist,
            tc.tile_pool(name="xs", bufs=2) as xs,
            tc.tile_pool(name="mo", bufs=4) as mo,
            tc.tile_pool(name="pp", bufs=6) as pp,
            tc.tile_pool(name="att", bufs=3) as att,
            tc.tile_pool(name="rbp", bufs=3) as rbp,
            tc.tile_pool(name="vts", bufs=2) as vts,
            tc.tile_pool(name="ost", bufs=4) as ost,
            tc.tile_pool(name="ps_proj", bufs=2, space="PSUM") as ps_proj_pool,
            tc.tile_pool(name="ps", bufs=2, space="PSUM") as ps_pool,
            tc.tile_pool(name="ps_pv", bufs=3, space="PSUM") as ps_pv_pool,
            tc.tile_pool(name="ps_l", bufs=1, space="PSUM") as ps_l_pool,
            tc.tile_pool(name="dram", bufs=4, space="DRAM") as dram,
        ):
            # ---- persistent SBUF state ----
            qt_sb = persist.tile([128, HPG, S], F16)       # QT per head [d, h, s]
            kt_sb = persist.tile([128, S], F16)            # KT          [d, s]
            v_sb = persist.tile([128, NKT, D], F16)        # V           [s, j, d]
            wq_sb = persist.tile([128, NKT, HPG * D], F16)
            wk_sb = persist.tile([128, NKT, D], F16)
            wv_sb = persist.tile([128, NKT, D], F16)
            wo_sb = persist.tile([128, NKT, CH], F16)
            cf16_sb = persist.tile([128, 257], F16)
            cf32_sb = persist.tile([128, NKT + 10], F32)
            nc.sync.dma_start(out=cf16_sb, in_=cf16.ap())
            nc.scalar.dma_start(out=cf32_sb, in_=cf32.ap())
            mask_sb = cf16_sb[:, 0:128]
            idr_sb = cf16_sb[:, 128:256]
            ones_sb = cf16_sb[:, 256:257]
            kb_sb = cf32_sb[:, 0:NKT]
            bq_sb = cf32_sb[:, NKT:NKT + 4]
            bk_sb = cf32_sb[:, NKT + 4:NKT + 5]
            bv_sb = cf32_sb[:, NKT + 5:NKT + 6]
            bo_sb = cf32_sb[:, NKT + 6:NKT + 10]

            # tiny warmup collective: absorbs rank launch skew + first-call
            # ncfw setup while the projections run, so the first real
            # AllGather runs at steady-state latency
            warm_in = dram.tile([128, 8], F16, name="warm_in")
            warm_out = dram.tile([G, 128, 8], F16, name="warm_out")
            nc.sync.dma_start(out=warm_in[:, 0:1], in_=ones_sb)
            if not sim_mode:
                nc.gpsimd.collective_compute(
                    "AllGather", mybir.AluOpType.bypass,
                    replica_groups=groups,
                    ins=[warm_in.opt()], outs=[warm_out.opt()],
                )

            # weights: (t p) n -> p t n so partition dim is hid-within-tile.
            # xt chunk 0 + per-k-tile weight transfers issue first and spread
            # across DMA queues so the first matmuls start within a few us.
            wq_r = wq.ap().rearrange("(t p) n -> p t n", p=128)
            wk_r = wk.ap().rearrange("(t p) n -> p t n", p=128)
            wv_r = wv.ap().rearrange("(t p) n -> p t n", p=128)
            wo_r = wo.ap().rearrange("(t p) n -> p t n", p=128)

            xt_r = xt.ap().rearrange("(t p) n -> p t n", p=128)

            def load_xt(c):
                xt_c = xs.tile([128, NKT, CH], F16, name="xt_c")
                sq = slice(c * CH, (c + 1) * CH)
                engs = [nc.sync, nc.sync, nc.sync, nc.sync]
                for u in range(4):
                    engs[u].dma_start(out=xt_c[:, 4 * u:4 * (u + 1), :],
                                      in_=xt_r[:, 4 * u:4 * (u + 1), sq])
                return xt_c

            xt_tiles = {0: load_xt(0)}
            for u in range(4):
                nc.gpsimd.dma_start(out=wq_sb[:, 4 * u:4 * (u + 1), :],
                                    in_=wq_r[:, 4 * u:4 * (u + 1), :])
            nc.gpsimd.dma_start(out=wk_sb, in_=wk_r)
            nc.gpsimd.dma_start(out=wv_sb, in_=wv_r)

            m_alls = []
            out_r = out.ap().rearrange("(t p) n -> p t n", p=128)

            def oproj(c):
                sq = slice(c * CH, (c + 1) * CH)
                m_all = m_alls[c]
                for t in range(HPG):
                    ps_o = ps_pool.tile([128, CH], F32, name="ps_any")
                    for ct in range(NKT):
                        nc.tensor.matmul(
                            ps_o, lhsT=wo_sb[:, ct, t * 128:(t + 1) * 128],
                            rhs=m_all[:, ct, :],
                            start=(ct == 0), stop=(ct == NKT - 1))
                    o_sb = ost.tile([128, CH], F16, name="o_sb")
                    nc.vector.tensor_scalar_add(o_sb, ps_o, bo_sb[:, t:t + 1])
                    eng = nc.sync if t % 2 == 0 else nc.gpsimd
                    eng.dma_start(out=out_r[:, t, sq], in_=o_sb)

            for c in range(NCH):
                sq = slice(c * CH, (c + 1) * CH)

                # ================= projections for chunk c =================
                # prefetch next chunk's xt ahead of this chunk's m_all loads
                if c + 1 < NCH:
                    xt_tiles[c + 1] = load_xt(c + 1)
                if c == 1:
                    nc.scalar.dma_start(out=wo_sb[:, :8, :], in_=wo_r[:, :8, :])
                    nc.scalar.dma_start(out=wo_sb[:, 8:, :], in_=wo_r[:, 8:, :])
                xt_c = xt_tiles[c]
                for h in range(HPG):
                    ps_q = ps_proj_pool.tile([128, CH], F32, name="ps_proj")
                    for t in range(NKT):
                        nc.tensor.matmul(ps_q,
                                         lhsT=wq_sb[:, t, h * D:(h + 1) * D],
                                         rhs=xt_c[:, t, :],
                                         start=(t == 0), stop=(t == NKT - 1))
                    nc.vector.tensor_scalar_add(qt_sb[:, h, sq], ps_q,
                                                bq_sb[:, h:h + 1])
                ps_k = ps_proj_pool.tile([128, CH], F32, name="ps_proj")
                for t in range(NKT):
                    nc.tensor.matmul(ps_k, lhsT=wk_sb[:, t, :], rhs=xt_c[:, t, :],
                                     start=(t == 0), stop=(t == NKT - 1))
                nc.vector.tensor_scalar_add(kt_sb[:, sq], ps_k, bk_sb)
                ps_v = ps_proj_pool.tile([128, CH], F32, name="ps_proj")
                for t in range(NKT):
                    nc.tensor.matmul(ps_v, lhsT=wv_sb[:, t, :], rhs=xt_c[:, t, :],
                                     start=(t == 0), stop=(t == NKT - 1))
                vt_t = vts.tile([128, CH], F16, name="vt_t")
                nc.vector.tensor_scalar_add(vt_t, ps_v, bv_sb)
                for u in range(4):
                    j = 4 * c + u
                    ps_tp = ps_pool.tile([128, 128], F16, name="ps_any")
                    nc.tensor.transpose(
                        ps_tp, vt_t[:, u * 128:(u + 1) * 128], idr_sb)
                    nc.scalar.activation(v_sb[:, j, :], ps_tp, Copy)

                # ================= attention for chunk c =================
                if c == NCH - 1:
                    m_all_last = mo.tile([128, NKT, CH], F16, name="m_all")
                # heads processed in pairs: shared KT_j/V_j stationary loads,
                # rowsums packed into one PE pass via col-tiling (M=1 each at
                # col-groups 0 and 32), one reciprocal per pair.
                njt = 4 * c + 4  # causal: key tiles 0..4c+3
                if c == NCH - 1:
                    ag_pair = [dram.tile([2, 128, CH], F16, name="ag_pa"),
                               dram.tile([2, 128, CH], F16, name="ag_pb")]
                else:
                    ag_in = dram.tile([HPG, 128, CH], F16, name="ag_in")
                for pr in range(HPG // 2):
                    ha, hb = 2 * pr, 2 * pr + 1
                    ps_pv_a = ps_pv_pool.tile([128, CH], F32, name="ps_pv")
                    ps_pv_b = ps_pv_pool.tile([128, CH], F32, name="ps_pv")
                    ps_l = ps_l_pool.tile([33, CH], F32, name="ps_l")
                    for j in range(njt):
                        # key tile j only reaches queries >= 128*r into the
                        # chunk (r = j - 4c >= 0 on the diagonal)
                        r = max(0, j - 4 * c)
                        cs = slice(128 * r, CH)
                        qs_ = slice(c * CH + 128 * r, (c + 1) * CH)
                        kjt = kt_sb[:, j * 128:(j + 1) * 128]
                        ps_s_a = ps_pool.tile([128, CH], F32, name="ps_any")
                        ps_s_b = ps_pool.tile([128, CH], F32, name="ps_any")
                        nc.tensor.matmul(ps_s_a[:, cs], lhsT=kjt,
                                         rhs=qt_sb[:, ha, qs_],
                                         start=True, stop=True)
                        nc.tensor.matmul(ps_s_b[:, cs], lhsT=kjt,
                                         rhs=qt_sb[:, hb, qs_],
                                         start=True, stop=True)
                        p_a = pp.tile([128, CH], F16, name="p_a")
                        p_b = pp.tile([128, CH], F16, name="p_b")
                        nc.scalar.activation(p_a[:, cs], ps_s_a[:, cs], Exp,
                                             scale=SCALE, bias=kb_sb[:, j:j + 1])
                        nc.scalar.activation(p_b[:, cs], ps_s_b[:, cs], Exp,
                                             scale=SCALE, bias=kb_sb[:, j:j + 1])
                        if j >= 4 * c:
                            # triangular mask on the 128-wide diagonal block
                            ds = slice(128 * r, 128 * (r + 1))
                            nc.vector.tensor_mul(p_a[:, ds], p_a[:, ds], mask_sb)
                            nc.vector.tensor_mul(p_b[:, ds], p_b[:, ds], mask_sb)
                        st, sp = (j == 0), (j == njt - 1)
                        nc.tensor.matmul(ps_pv_a[:, cs], lhsT=v_sb[:, j, :],
                                         rhs=p_a[:, cs], start=st, stop=sp)
                        nc.tensor.matmul(ps_pv_b[:, cs], lhsT=v_sb[:, j, :],
                                         rhs=p_b[:, cs], start=st, stop=sp)
                        # two interleaved rowsum accumulation groups share one
                        # PSUM bank (partitions 0 / 32); col-tiled so the two
                        # M=1 matmuls run concurrently in the PE array.
                        nc.tensor.matmul(ps_l[0:1, cs], lhsT=ones_sb,
                                         rhs=p_a[:, cs], start=st, stop=sp,
                                         tile_position=(0, 0),
                                         skip_group_check=True)
                        nc.tensor.matmul(ps_l[32:33, cs], lhsT=ones_sb,
                                         rhs=p_b[:, cs], start=st, stop=sp,
                                         tile_position=(0, 32),
                                         skip_group_check=True)
                    # partition_broadcast requires its input at partition 0,
                    # so each reciprocal lands in its own partition-0 tile
                    rl_a = rbp.tile([1, CH], F32, name="rl_a")
                    rl_b = rbp.tile([1, CH], F32, name="rl_b")
                    nc.vector.reciprocal(rl_a, ps_l[0:1, :])
                    nc.vector.reciprocal(rl_b, ps_l[32:33, :])
                    rb_a = rbp.tile([128, CH], F32, name="rb_a")
                    rb_b = rbp.tile([128, CH], F32, name="rb_b")
                    nc.gpsimd.partition_broadcast(rb_a, rl_a, channels=128)
                    nc.gpsimd.partition_broadcast(rb_b, rl_b, channels=128)
                    at_a = att.tile([128, CH], F16, name="at_sb")
                    at_b = att.tile([128, CH], F16, name="at_sb")
                    nc.vector.tensor_mul(at_a, ps_pv_a, rb_a)
                    nc.vector.tensor_mul(at_b, ps_pv_b, rb_b)
                    if c == NCH - 1:
                        nc.scalar.dma_start(out=ag_pair[pr][0], in_=at_a)
                        nc.scalar.dma_start(out=ag_pair[pr][1], in_=at_b)
                        ago = dram.tile([G, 2, 128, CH], F16, name="ag_po")
                        if sim_mode:
                            for g in range(G):
                                nc.sync.dma_start(out=ago[g], in_=ag_pair[pr][:])
                        else:
                            nc.gpsimd.collective_compute(
                                "AllGather", mybir.AluOpType.bypass,
                                replica_groups=groups,
                                ins=[ag_pair[pr].opt()], outs=[ago.opt()],
                            )
                        agr = ago.rearrange("g h p n -> p (g h) n")
                        m3 = m_all_last
                        for g in range(G):
                            eng = nc.scalar if g % 2 == 0 else nc.gpsimd
                            eng.dma_start(
                                out=m3[:, 4 * g + 2 * pr:4 * g + 2 * pr + 2, :],
                                in_=agr[:, 2 * g:2 * g + 2, :])
                    else:
                        nc.scalar.dma_start(out=ag_in[ha], in_=at_a)
                        nc.scalar.dma_start(out=ag_in[hb], in_=at_b)

                if c == NCH - 1:
                    m_alls.append(m_all_last)
                else:
                    ag_out = dram.tile([G, HPG, 128, CH], F16, name="ag_out")
                    if sim_mode:
                        # stand-in for the AllGather with equivalent local IO
                        for g in range(G):
                            nc.sync.dma_start(out=ag_out[g], in_=ag_in[:])
                    else:
                        nc.gpsimd.collective_compute(
                            "AllGather", mybir.AluOpType.bypass,
                            replica_groups=groups,
                            ins=[ag_in.opt()], outs=[ag_out.opt()],
                        )
                    m_all = mo.tile([128, NKT, CH], F16, name="m_all")
                    ag_r = ag_out.rearrange("g h p n -> p (g h) n")
                    nc.scalar.dma_start(out=m_all[:, :8, :], in_=ag_r[:, :8, :])
                    nc.gpsimd.dma_start(out=m_all[:, 8:, :], in_=ag_r[:, 8:, :])
                    m_alls.append(m_all)

            # ================= O-projections =================
            for c in range(NCH):
                oproj(c)

    nc.compile()
    return nc


def _host_consts():
    # f16 blob: [triangular mask | identity | ones]; mask[p, col] = col >= p
    col = np.arange(128)[None, :]
    p = np.arange(128)[:, None]
    maskt = (col >= p).astype(np.float16)
    ident = np.eye(128, dtype=np.float16)
    ones = np.ones((128, 1), dtype=np.float16)
    return np.ascontiguousarray(np.concatenate([maskt, ident, ones], axis=1))


def kernel(hidden_states, attention_mask, Wq, bq, Wk, bk, Wv, bv, Wo, bo):
    from concourse.bass_utils import run_bass_kernel_spmd

    global _CACHED_NC
    if _CACHED_NC is None:
        _CACHED_NC = _build_nc()
    nc = _CACHED_NC

    X = np.asarray(hidden_states, dtype=np.float32).astype(np.float16)
    am = np.asarray(attention_mask).astype(np.float32)  # [B, S] key mask
    Wq16 = np.asarray(Wq, np.float32).astype(np.float16)
    Wk16 = np.asarray(Wk, np.float32).astype(np.float16)
    Wv16 = np.asarray(Wv, np.float32).astype(np.float16)
    Wo16 = np.asarray(Wo, np.float32).astype(np.float16)
    cf16_host = _host_consts()

    in_maps = []
    for c in range(8):
        b, g = divmod(c, G)
        qs = slice(g * HPG * D, (g + 1) * HPG * D)   # q-head cols of group g
        ks = slice(g * D, (g + 1) * D)               # kv-head cols of group g
        in_maps.append({
            "xt": np.ascontiguousarray(X[b].T),
            "wq": np.ascontiguousarray(Wq16[:, qs]),
            "wk": np.ascontiguousarray(Wk16[:, ks]),
            "wv": np.ascontiguousarray(Wv16[:, ks]),
            "wo": np.ascontiguousarray(Wo16[:, qs]),  # hid cols [512g, 512g+512)
            "cf16": cf16_host.copy(),
            "cf32": np.ascontiguousarray(np.concatenate([
                ((1.0 - am[b]) * -10000.0).astype(np.float32)
                .reshape(NKT, 128).T,
                np.asarray(bq, np.float32)[qs].reshape(HPG, D).T,
                np.asarray(bk, np.float32)[ks].reshape(D, 1),
                np.asarray(bv, np.float32)[ks].reshape(D, 1),
                np.asarray(bo, np.float32)[qs].reshape(HPG, D).T,
            ], axis=1)),
        })

    global _last_in_maps
    _last_in_maps = in_maps
    res = run_bass_kernel_spmd(nc, in_maps, core_ids=list(range(8)))
    out = np.empty((B, S, HID), dtype=np.float32)
    for c in range(8):
        b, g = divmod(c, G)
        out[b][:, g * CH:(g + 1) * CH] = res.results[c]["out"].T.astype(np.float32)
    return out
